# revision 1
# baseline (speedup 1.0000x reference)
"""CGC layer (gated graph conv message passing) on 8 trn2 NeuronCores.

Math (per edge e with sender s, receiver r):
    c    = [x[s], x[r], ef[e]]                  # [320]
    vals = softplus(c @ W_val.T + b_val)        # [128]
    gate = sigmoid (c @ W_mul.T + b_mul)        # [128]
    out[r] += vals * gate                       # segment-sum over receivers

Strategy (edge-parallel, receiver-sharded so no cross-core reduction):
  * Host: sort edges by receiver; partition nodes into 8 contiguous ranges
    of 49 blocks x 128 nodes; pad each 128-node block's edge list to K
    chunks of 128 edges (K uniform across all blocks/cores -> one SPMD
    program).  Pre-gather x[s]/x[r] rows into edge-aligned fp16 streams,
    stored feature-major ([128, E_pad]) so they are directly usable as
    matmul stationary operands (sharding hint: "shard edges (edge_index,
    edge_ft, gathered rows)").
  * Device per chunk of 128 edges: 3 matmuls with fused [val|mul] weights
    (N=256) accumulate A|B in PSUM; ACT (single natural_log_exp table set)
    computes t=exp(A), u=exp(-B), vals=ln(t+1); DVE computes
    msg = vals / (1+u) = softplus(A)*sigmoid(B); GpSimd builds the one-hot
    selector sel[e,n] = (rcv_loc[e]==n); PE scatter-adds via
    psum_out[n,f] += sel.T @ msg per 128-node block.
  * Output blocks DMA to DRAM; host concatenates core outputs.
"""

import os
import sys

sys.path.insert(0, "/opt/trn_rl_repo")

import numpy as np

from concourse import bacc, bass, mybir, tile
from concourse.bass_utils import run_bass_kernel_spmd

N_CORES = 8
P = 128            # partition / chunk size
G = 6              # chunks per PSUM group
NODE_DIM = 128
EDGE_DIM = 64
F16 = mybir.dt.float16
F32 = mybir.dt.float32

# gate path: "divide" uses AluOpType.divide on DVE (verify vs HW!),
# "recip" uses reciprocal_approx_fast (fp32, known-good).
GATE_MODE = os.environ.get("CGC_GATE_MODE", "recip")
# debug bisection: "s1" = skip Ln+gate (msg:=t), "s2" = skip gate (msg:=vals),
# "full" = everything
STAGE = os.environ.get("CGC_STAGE", "full")


# ----------------------------------------------------------------- host prep
def _preprocess(x, edge_index, edge_ft, W_val, b_val, W_mul, b_mul):
    n_nodes = x.shape[0]
    snd = np.asarray(edge_index[0], dtype=np.int64)
    rcv = np.asarray(edge_index[1], dtype=np.int64)

    blocks_per_core = int(np.ceil(n_nodes / (N_CORES * P)))  # 49 for 50000
    nodes_per_core = blocks_per_core * P                     # 6272
    n_blocks = N_CORES * blocks_per_core

    order = np.argsort(rcv, kind="stable")
    rcv_s = rcv[order]
    snd_s = snd[order]

    block_id = (rcv_s // P).astype(np.int64)
    counts = np.bincount(block_id, minlength=n_blocks)
    k_chunks = int(np.ceil(counts.max() / P))
    k_chunks = max(G, int(np.ceil(k_chunks / G)) * G)        # multiple of G
    e_pad = blocks_per_core * k_chunks * P                   # per core

    # destination slot for every sorted edge
    block_start_in = np.zeros(n_blocks + 1, dtype=np.int64)
    np.cumsum(counts, out=block_start_in[1:])
    within = np.arange(len(rcv_s), dtype=np.int64) - block_start_in[block_id]
    core_of = block_id // blocks_per_core
    blk_in_core = block_id % blocks_per_core
    dst = blk_in_core * (k_chunks * P) + within              # slot within core

    snd_pad = np.zeros((N_CORES, e_pad), dtype=np.int64)
    rcv_pad = np.zeros((N_CORES, e_pad), dtype=np.int64)
    eid_pad = np.zeros((N_CORES, e_pad), dtype=np.int64)     # sorted-edge id, -1 pad
    rloc_pad = np.full((N_CORES, e_pad), 999.0, dtype=np.float32)
    valid = np.zeros((N_CORES, e_pad), dtype=bool)

    snd_pad[core_of, dst] = snd_s
    rcv_pad[core_of, dst] = rcv_s
    eid_pad[core_of, dst] = order
    rloc_pad[core_of, dst] = (rcv_s % P).astype(np.float16)
    valid[core_of, dst] = True

    xf = np.asarray(x, dtype=np.float32)
    eff = np.asarray(edge_ft, dtype=np.float32)

    xsT = np.empty((N_CORES, NODE_DIM, e_pad), dtype=np.float16)
    xrT = np.empty((N_CORES, NODE_DIM, e_pad), dtype=np.float16)
    efT = np.empty((N_CORES, EDGE_DIM + 1, e_pad), dtype=np.float16)
    for c in range(N_CORES):
        xsT[c] = xf[snd_pad[c]].T.astype(np.float16)
        xrT[c] = xf[rcv_pad[c]].T.astype(np.float16)
        ef_rows = np.where(valid[c][:, None], eff[eid_pad[c]], 0.0)
        efT[c, :EDGE_DIM] = ef_rows.T.astype(np.float16)
        efT[c, EDGE_DIM] = 1.0                                # bias row

    # RLT[p, b*K + cch] = rcv_loc of edge (block b, chunk cch, slot p)
    RLT = np.ascontiguousarray(
        rloc_pad.reshape(N_CORES, blocks_per_core * k_chunks, P).transpose(0, 2, 1)
    ).astype(np.float16)

    # B(gate) half negated on host: psum accumulates [A | -B], so one Exp
    # pass yields [e^A | e^-B].
    Wv = np.asarray(W_val, dtype=np.float32)
    Wm = -np.asarray(W_mul, dtype=np.float32)
    bv = np.asarray(b_val, dtype=np.float32)
    bm = -np.asarray(b_mul, dtype=np.float32)
    Wsnd = np.concatenate([Wv[:, :128].T, Wm[:, :128].T], axis=1).astype(np.float16)
    Wrcv = np.concatenate([Wv[:, 128:256].T, Wm[:, 128:256].T], axis=1).astype(
        np.float16
    )
    Wef = np.concatenate(
        [
            np.concatenate([Wv[:, 256:320].T, Wm[:, 256:320].T], axis=1),
            np.concatenate([bv, bm])[None, :],
        ],
        axis=0,
    ).astype(np.float16)

    return dict(
        xsT=xsT, xrT=xrT, efT=efT, RLT=RLT,
        Wsnd=Wsnd, Wrcv=Wrcv, Wef=Wef,
        blocks_per_core=blocks_per_core, k_chunks=k_chunks, e_pad=e_pad,
        nodes_per_core=nodes_per_core,
    )


# ------------------------------------------------------------- device kernel
def _build_nc(blocks_per_core, k_chunks, e_pad):
    B, K = blocks_per_core, k_chunks
    n_groups = K // G
    nc = bacc.Bacc("TRN2", target_bir_lowering=False, debug=False)

    xsT = nc.dram_tensor("xsT", [NODE_DIM, e_pad], F16, kind="ExternalInput")
    xrT = nc.dram_tensor("xrT", [NODE_DIM, e_pad], F16, kind="ExternalInput")
    efT = nc.dram_tensor("efT", [EDGE_DIM + 1, e_pad], F16, kind="ExternalInput")
    RLT = nc.dram_tensor("RLT", [P, B * K], F16, kind="ExternalInput")
    Wsnd_d = nc.dram_tensor("Wsnd", [NODE_DIM, 256], F16, kind="ExternalInput")
    Wrcv_d = nc.dram_tensor("Wrcv", [NODE_DIM, 256], F16, kind="ExternalInput")
    Wef_d = nc.dram_tensor("Wef", [EDGE_DIM + 1, 256], F16, kind="ExternalInput")
    out_d = nc.dram_tensor("out", [B * P, NODE_DIM], F32, kind="ExternalOutput")
    debug = os.environ.get("CGC_DEBUG") == "1"
    if debug:
        dbg_tu = nc.dram_tensor("dbg_tu", [P, K, 256], F16, kind="ExternalOutput")
        dbg_vals = nc.dram_tensor("dbg_vals", [P, K, P], F16, kind="ExternalOutput")
        dbg_msg = nc.dram_tensor("dbg_msg", [P, K, P], F16, kind="ExternalOutput")
        dbg_sel = nc.dram_tensor("dbg_sel", [P, K, P], F16, kind="ExternalOutput")
        dbg_rl = nc.dram_tensor("dbg_rl", [P, K], F16, kind="ExternalOutput")

    with tile.TileContext(nc) as tc:
        with (
            tc.tile_pool(name="const", bufs=1) as cpool,
            tc.tile_pool(name="stream", bufs=3) as spool,
            tc.tile_pool(name="blk", bufs=2) as bpool,
            tc.tile_pool(name="psumAB", bufs=2, space="PSUM") as ppool,
            tc.tile_pool(name="psumOut", bufs=2, space="PSUM") as opool,
        ):
            # constants
            w_snd = cpool.tile([NODE_DIM, 256], F16)
            w_rcv = cpool.tile([NODE_DIM, 256], F16)
            w_ef = cpool.tile([EDGE_DIM + 1, 256], F16)
            nc.sync.dma_start(out=w_snd[:], in_=Wsnd_d[:])
            nc.sync.dma_start(out=w_rcv[:], in_=Wrcv_d[:])
            nc.sync.dma_start(out=w_ef[:], in_=Wef_d[:])
            # iota_rep[p, k, j] = j  (repeated along chunks) for the batched
            # selector compare
            iota_rep = cpool.tile([P, K, P], F16)
            nc.gpsimd.iota(
                iota_rep[:], pattern=[[0, K], [1, P]], base=0,
                channel_multiplier=0,
                allow_small_or_imprecise_dtypes=True,
            )

            for b in range(B):
                rl = bpool.tile([P, K], F16, tag="rl")
                nc.sync.dma_start(out=rl[:], in_=RLT[:, b * K:(b + 1) * K])

                tu = bpool.tile([P, K, 256], F16, tag="tu")
                sel = bpool.tile([P, K, P], F16, tag="sel")

                for g in range(n_groups):
                    off = b * K * P + g * G * P
                    xs_g = spool.tile([NODE_DIM, G * P], F16, tag="xs")
                    xr_g = spool.tile([NODE_DIM, G * P], F16, tag="xr")
                    ef_g = spool.tile([EDGE_DIM + 1, G * P], F16, tag="ef")
                    nc.sync.dma_start(out=xs_g[:], in_=xsT[:, off:off + G * P])
                    nc.sync.dma_start(out=xr_g[:], in_=xrT[:, off:off + G * P])
                    nc.sync.dma_start(out=ef_g[:], in_=efT[:, off:off + G * P])

                    pab = ppool.tile([P, G, 256], F32, tag="ab")
                    for q in range(G):
                        sl = slice(q * P, (q + 1) * P)
                        nc.tensor.matmul(
                            out=pab[:, q, :], lhsT=xs_g[:, sl], rhs=w_snd[:],
                            start=True, stop=False,
                        )
                        nc.tensor.matmul(
                            out=pab[:, q, :], lhsT=xr_g[:, sl], rhs=w_rcv[:],
                            start=False, stop=False,
                        )
                        nc.tensor.matmul(
                            out=pab[:, q, :], lhsT=ef_g[:, sl], rhs=w_ef[:],
                            start=False, stop=True,
                        )

                    # [t | u] = exp([A | -B]) (B pre-negated)
                    gsl = slice(g * G, (g + 1) * G)
                    if os.environ.get("CGC_EXP_MODE", "fused") == "fused":
                        nc.scalar.activation(
                            out=tu[:, gsl, :], in_=pab[:],
                            func=mybir.ActivationFunctionType.Exp,
                        )
                    else:
                        nc.scalar.activation(
                            out=tu[:, gsl, 0:P], in_=pab[:, :, 0:P],
                            func=mybir.ActivationFunctionType.Exp,
                        )
                        nc.scalar.activation(
                            out=tu[:, gsl, P:256], in_=pab[:, :, P:256],
                            func=mybir.ActivationFunctionType.Exp,
                        )

                # selector: sel[p, k, j] = (rl[p, k] == j)
                if os.environ.get("CGC_SEL_MODE", "batched") == "batched":
                    nc.vector.tensor_tensor(
                        out=sel[:],
                        in0=iota_rep[:],
                        in1=rl[:].to_broadcast([P, K, P]),
                        op=mybir.AluOpType.is_equal,
                    )
                else:
                    for qg in range(K):
                        nc.vector.tensor_tensor(
                            out=sel[:, qg, :],
                            in0=iota_rep[:, qg, :],
                            in1=rl[:, qg:qg + 1].to_broadcast([P, P]),
                            op=mybir.AluOpType.is_equal,
                        )
                # vals = ln(t + 1)  [softplus]
                vals = bpool.tile([P, K, P], F16, tag="vals")
                nc.scalar.activation(
                    out=vals[:], in_=tu[:, :, 0:P],
                    func=mybir.ActivationFunctionType.Ln, bias=1.0,
                )
                # gate = 1/(1+u); msg = vals*gate
                msg = bpool.tile([P, K, P], F16, tag="msg")
                w32 = bpool.tile([P, K, P], F32, tag="w32")
                g32 = bpool.tile([P, K, P], F32, tag="g32")
                nc.vector.tensor_scalar_add(w32[:], tu[:, :, P:256], 1.0)
                if GATE_MODE == "recip_exact":
                    nc.vector.reciprocal(out=g32[:], in_=w32[:])
                else:
                    nc.vector.reciprocal_approx_fast(out=g32[:], in_=w32[:])
                nc.vector.tensor_tensor(
                    out=msg[:], in0=vals[:], in1=g32[:],
                    op=mybir.AluOpType.mult,
                )

                if debug and b == 0:
                    nc.sync.dma_start(out=dbg_tu[:], in_=tu[:])
                    nc.sync.dma_start(out=dbg_vals[:], in_=vals[:])
                    nc.sync.dma_start(out=dbg_msg[:], in_=msg[:])
                    nc.sync.dma_start(out=dbg_sel[:], in_=sel[:])
                    nc.sync.dma_start(out=dbg_rl[:], in_=rl[:])

                pout = opool.tile([P, P], F32, tag="out")
                for qg in range(K):
                    nc.tensor.matmul(
                        out=pout[:], lhsT=sel[:, qg, :], rhs=msg[:, qg, :],
                        start=(qg == 0), stop=(qg == K - 1),
                    )
                o_sb = bpool.tile([P, P], F32, tag="osb")
                nc.vector.tensor_copy(out=o_sb[:], in_=pout[:])
                nc.sync.dma_start(out=out_d[b * P:(b + 1) * P, :], in_=o_sb[:])

    nc.compile()
    return nc


# ------------------------------------------------------------------ entry
def kernel(x, edge_index, edge_ft, W_val, b_val, W_mul, b_mul, _trace=False):
    n_nodes = x.shape[0]
    prep = _preprocess(x, edge_index, edge_ft, W_val, b_val, W_mul, b_mul)
    nc = _build_nc(prep["blocks_per_core"], prep["k_chunks"], prep["e_pad"])

    in_maps = []
    for c in range(N_CORES):
        in_maps.append(
            {
                "xsT": prep["xsT"][c], "xrT": prep["xrT"][c],
                "efT": prep["efT"][c], "RLT": prep["RLT"][c],
                "Wsnd": prep["Wsnd"], "Wrcv": prep["Wrcv"], "Wef": prep["Wef"],
            }
        )
    res = run_bass_kernel_spmd(nc, in_maps, list(range(N_CORES)), trace=_trace)
    outs = [np.asarray(res.results[c]["out"]) for c in range(N_CORES)]
    full = np.concatenate(outs, axis=0)[:n_nodes]
    if _trace:
        return full.astype(np.float32), res
    return full.astype(np.float32)

